# revision 6
# baseline (speedup 1.0000x reference)
"""Multi-head cross-attention (B=4, S=2048, D=1024, H=16) on 8 Trainium2 cores.

Sharding: hybrid data/tensor parallel. Core c handles batch b = c//2 and
head-group g = c%2 (8 of the 16 heads, i.e. 512 of the 1024 q/k/v dims).
Each core computes a partial out-projection over its 512 attention dims;
the host sums the two partials per batch (the "all-reduce after out_lin"
of the tensor-parallel split, done on host since pairs share a batch).

Per-core kernel (all matmuls in float32r = rounded-fp32 PE fast path):
  Q.T = wq_t.T @ x_t   (+bq)      [512, 2048]   (o on partitions)
  K.T = wk_t.T @ mem_t (+bk)      [512, 2048]
  V   = mem_t.T @ wv_t            [2048, 512] stored as v_aug [*, 8, 65]
                                  with a ones column per head (denominator)
  per head h, query-half qh:
    S.T[k,q] = K_h @ Q_h.T                (K=64 contraction)
    P.T      = exp(0.125*S.T + mask_bias) (ACT, bias is per-key partition)
    AV.T     = [V_h|1].T @ P.T  -> [65, 1024] PSUM accum over k-chunks
    attn.T   = AV.T[0:64] * recip(AV.T[64])  (Newton-refined reciprocal)
  out.T = wo_t.T @ attn.T (+bo_eff on core g=0)  [1024, 2048] partial

bv is folded into bo on the host: out = attn@wo.T + (bo + wo@bv) because
softmax rows sum to 1. The key-padding mask enters as an additive
per-partition bias in the exp activation (exact, and free).
"""

import numpy as np

import concourse.bacc as bacc
import concourse.mybir as mybir
from concourse import tile
from concourse.bass_utils import run_bass_kernel_spmd

F32 = mybir.dt.float32
F32R = mybir.dt.float32r
AF = mybir.ActivationFunctionType

B, S, D = 4, 2048, 1024
H, HD = 16, 64
NCORES = 8
NH = 8          # heads per core
OD = NH * HD    # 512 attention dims per core
P = 128
NDC = D // P    # 8 d-chunks
NKC = S // P    # 16 key chunks
NEG = -1.0e30

_cache = {}


def _build():
    nc = bacc.Bacc(None, target_bir_lowering=False, debug=False)

    x_t = nc.dram_tensor("x_t", [D, S], F32R, kind="ExternalInput").ap()
    mem_t = nc.dram_tensor("mem_t", [D, S], F32R, kind="ExternalInput").ap()
    wq_t = nc.dram_tensor("wq_t", [D, OD], F32R, kind="ExternalInput").ap()
    wk_t = nc.dram_tensor("wk_t", [D, OD], F32R, kind="ExternalInput").ap()
    wv_t = nc.dram_tensor("wv_t", [D, OD], F32R, kind="ExternalInput").ap()
    wo_t = nc.dram_tensor("wo_t", [OD, D], F32R, kind="ExternalInput").ap()
    bq_s = nc.dram_tensor("bq_s", [P, OD // P], F32, kind="ExternalInput").ap()
    bk_s = nc.dram_tensor("bk_s", [P, OD // P], F32, kind="ExternalInput").ap()
    bo_s = nc.dram_tensor("bo_s", [P, D // P], F32, kind="ExternalInput").ap()
    maskb = nc.dram_tensor("maskb", [P, NKC], F32, kind="ExternalInput").ap()
    out_t = nc.dram_tensor("out_t", [D, S], F32, kind="ExternalOutput").ap()

    x_c = x_t.rearrange("(c p) s -> c p s", p=P)
    m_c = mem_t.rearrange("(c p) s -> c p s", p=P)
    wq_c = wq_t.rearrange("(c p) o -> c p o", p=P)
    wk_c = wk_t.rearrange("(c p) o -> c p o", p=P)
    wv_c = wv_t.rearrange("(c p) o -> c p o", p=P)
    wo_c = wo_t.rearrange("(c p) o -> c p o", p=P)

    from contextlib import ExitStack
    with tile.TileContext(nc) as tc, ExitStack() as ctx:
        q_pool = ctx.enter_context(tc.tile_pool(name="qt", bufs=1))
        k_pool = ctx.enter_context(tc.tile_pool(name="kt", bufs=1))
        v_pool = ctx.enter_context(tc.tile_pool(name="va", bufs=1))
        a_pool = ctx.enter_context(tc.tile_pool(name="at", bufs=1))
        c_pool = ctx.enter_context(tc.tile_pool(name="cst", bufs=1))
        proj_ctx = ExitStack()
        xm_pool = proj_ctx.enter_context(tc.tile_pool(name="xm", bufs=8))
        w_pool = proj_ctx.enter_context(tc.tile_pool(name="wt", bufs=3))
        if True:
            # ---- constants ----
            bq_sb = c_pool.tile([P, OD // P], F32, tag="bq")
            bk_sb = c_pool.tile([P, OD // P], F32, tag="bk")
            bo_sb = c_pool.tile([P, D // P], F32, tag="bo")
            mk_sb = c_pool.tile([P, NKC], F32, tag="mk")
            nc.sync.dma_start(out=bq_sb[:], in_=bq_s[:])
            nc.sync.dma_start(out=bk_sb[:], in_=bk_s[:])
            nc.sync.dma_start(out=bo_sb[:], in_=bo_s[:])
            nc.sync.dma_start(out=mk_sb[:], in_=maskb[:])
            ones_f = c_pool.tile([P, NH], F32, tag="onef")
            nc.vector.memset(ones_f[:], 1.0)
            ones_r = c_pool.tile([P, NH], F32R, tag="oner")
            nc.vector.tensor_copy(ones_r[:], ones_f[:])

            qT = [q_pool.tile([P, S], F32R, tag=f"q{m}", name=f"q{m}")
                  for m in range(OD // P)]
            kT = [k_pool.tile([P, S], F32R, tag=f"k{m}", name=f"k{m}")
                  for m in range(OD // P)]
            v_aug = [v_pool.tile([P, NH, 65], F32R, tag=f"v{st}", name=f"v{st}")
                     for st in range(NKC)]

            HS = S // 2  # column-block of 1024 tokens
            with tc.tile_pool(name="pj", bufs=4, space="PSUM") as pj_pool:
                # ---- phases 1+2+3 blocked over token halves ----
                for nb in range(2):
                    c_sl = slice(nb * HS, (nb + 1) * HS)
                    # phase 1: Q.T over this query half
                    x_tiles = []
                    for i in range(NDC):
                        t = xm_pool.tile([P, HS], F32R, tag="xm", name="xt")
                        nc.sync.dma_start(out=t[:], in_=x_c[i, :, c_sl])
                        x_tiles.append(t)
                    for m in range(OD // P):
                        wq_tiles = []
                        for i in range(NDC):
                            wt = w_pool.tile([P, P], F32R, tag="w", name="wqt", bufs=10)
                            nc.sync.dma_start(
                                out=wt[:], in_=wq_c[i, :, m * P:(m + 1) * P])
                            wq_tiles.append(wt)
                        for n in range(HS // 512):
                            ps = pj_pool.tile([P, 512], F32, tag="pj", name="psq")
                            for i in range(NDC):
                                nc.tensor.matmul(
                                    ps[:], wq_tiles[i][:],
                                    x_tiles[i][:, n * 512:(n + 1) * 512],
                                    start=(i == 0), stop=(i == NDC - 1),
                                )
                            nc.vector.tensor_scalar_add(
                                qT[m][:, nb * HS + n * 512:nb * HS + (n + 1) * 512],
                                ps[:], bq_sb[:, m:m + 1],
                            )
                    # phase 2: K.T over this key half
                    m_tiles = []
                    for i in range(NDC):
                        t = xm_pool.tile([P, HS], F32R, tag="xm", name="mt")
                        nc.sync.dma_start(out=t[:], in_=m_c[i, :, c_sl])
                        m_tiles.append(t)
                    for m in range(OD // P):
                        wk_tiles = []
                        for i in range(NDC):
                            wt = w_pool.tile([P, P], F32R, tag="w", name="wkt", bufs=10)
                            nc.sync.dma_start(
                                out=wt[:], in_=wk_c[i, :, m * P:(m + 1) * P])
                            wk_tiles.append(wt)
                        for n in range(HS // 512):
                            ps = pj_pool.tile([P, 512], F32, tag="pj", name="psk")
                            for i in range(NDC):
                                nc.tensor.matmul(
                                    ps[:], wk_tiles[i][:],
                                    m_tiles[i][:, n * 512:(n + 1) * 512],
                                    start=(i == 0), stop=(i == NDC - 1),
                                )
                            nc.vector.tensor_scalar_add(
                                kT[m][:, nb * HS + n * 512:nb * HS + (n + 1) * 512],
                                ps[:], bk_sb[:, m:m + 1],
                            )
                    # phase 3: V for this key half
                    wv_tiles = []
                    for i in range(NDC):
                        wt = w_pool.tile([P, OD], F32R, tag="wv", name="wvt", bufs=8)
                        nc.sync.dma_start(out=wt[:], in_=wv_c[i])
                        wv_tiles.append(wt)
                    for sti in range(HS // P):
                        st = nb * (HS // P) + sti
                        ps = pj_pool.tile([P, 512], F32, tag="pj", name="psv")
                        for i in range(NDC):
                            nc.tensor.matmul(
                                ps[:], m_tiles[i][:, sti * P:(sti + 1) * P],
                                wv_tiles[i][:],
                                start=(i == 0), stop=(i == NDC - 1),
                            )
                        nc.vector.tensor_copy(
                            v_aug[st][:, :, 0:64],
                            ps[:].rearrange("p (h d) -> p h d", h=NH),
                        )
                        nc.vector.tensor_copy(
                            v_aug[st][:, :, 64:65], ones_r[:].unsqueeze(2))

            proj_ctx.close()

            # ---- phase 4: attention, head-by-head ----
            attn = [a_pool.tile([P, S], F32R, tag=f"a{m}", name=f"a{m}") for m in range(OD // P)]
            with (
                tc.tile_pool(name="es", bufs=3) as e_pool,
                tc.tile_pool(name="nrm", bufs=1) as n_pool,
                tc.tile_pool(name="lg", bufs=2, space="PSUM") as lg_pool,
                tc.tile_pool(name="av", bufs=2, space="PSUM") as av_pool,
            ):
                for qh in range(2):
                    q_sl = slice(qh * 1024, (qh + 1) * 1024)
                    for h in range(NH):
                        mt, ro = h // 2, 64 * (h % 2)
                        av = av_pool.tile([65, 1024], F32, tag="av")
                        for kc in range(NKC):
                            lg = lg_pool.tile([P, 1024], F32, tag="lg")
                            for n in range(2):
                                nc.tensor.matmul(
                                    lg[:, n * 512:(n + 1) * 512],
                                    kT[mt][ro:ro + 64, kc * P:(kc + 1) * P],
                                    qT[mt][ro:ro + 64,
                                           qh * 1024 + n * 512:
                                           qh * 1024 + (n + 1) * 512],
                                    start=True, stop=True,
                                )
                            es = e_pool.tile([P, 1024], F32R, tag="es")
                            nc.scalar.activation(
                                es[:], lg[:], AF.Exp,
                                bias=mk_sb[:, kc:kc + 1], scale=0.125,
                            )
                            for n in range(2):
                                nc.tensor.matmul(
                                    av[:, n * 512:(n + 1) * 512],
                                    v_aug[kc][:, h, :],
                                    es[:, n * 512:(n + 1) * 512],
                                    start=(kc == 0), stop=(kc == NKC - 1),
                                )
                        # normalize: attn = av[0:64] * recip(av[64])
                        r0 = n_pool.tile([1, 1024], F32, tag="r0")
                        t1 = n_pool.tile([1, 1024], F32, tag="t1")
                        bc = n_pool.tile([64, 1024], F32, tag="bc")
                        nc.vector.reciprocal(r0[:], av[64:65, :])
                        nc.vector.tensor_mul(t1[:], av[64:65, :], r0[:])
                        nc.vector.tensor_scalar(
                            t1[:], t1[:], -1.0, 2.0,
                            mybir.AluOpType.mult, mybir.AluOpType.add,
                        )
                        nc.vector.tensor_mul(r0[:], r0[:], t1[:])
                        nc.gpsimd.partition_broadcast(bc[:], r0[:])
                        nc.vector.tensor_mul(
                            attn[mt][ro:ro + 64, q_sl], av[0:64, :], bc[:],
                        )

            # ---- phase 5: out.T = wo_t.T @ attn.T (+bo_eff) ----
            with (
                tc.tile_pool(name="wt2", bufs=3) as w_pool,
                tc.tile_pool(name="ev", bufs=3) as o_pool,
                tc.tile_pool(name="po", bufs=4, space="PSUM") as po_pool,
            ):
                for m in range(D // P):
                    wo_tiles = []
                    for i in range(OD // P):
                        wt = w_pool.tile([P, P], F32R, tag="w", bufs=6)
                        nc.sync.dma_start(out=wt[:], in_=wo_c[i, :, m * P:(m + 1) * P])
                        wo_tiles.append(wt)
                    for n in range(S // 512):
                        ps = po_pool.tile([P, 512], F32, tag="po")
                        for i in range(OD // P):
                            nc.tensor.matmul(
                                ps[:], wo_tiles[i][:],
                                attn[i][:, n * 512:(n + 1) * 512],
                                start=(i == 0), stop=(i == OD // P - 1),
                            )
                        ev = o_pool.tile([P, 512], F32, tag="ev")
                        nc.vector.tensor_scalar_add(ev[:], ps[:], bo_sb[:, m:m + 1])
                        nc.sync.dma_start(
                            out=out_t[m * P:(m + 1) * P, n * 512:(n + 1) * 512],
                            in_=ev[:],
                        )

    nc.compile()
    return nc


def _prep_inputs(x, memory, mask, wq, bq, wk, bk, wv, bv, wo, bo):
    f = np.float32
    wqT = np.ascontiguousarray(wq.T, dtype=f)
    wkT = np.ascontiguousarray(wk.T, dtype=f)
    wvT = np.ascontiguousarray(wv.T, dtype=f)
    woT = np.ascontiguousarray(wo.T, dtype=f)
    bo_eff = (bo.astype(f) + wo.astype(f) @ bv.astype(f))
    zeros_bo = np.zeros_like(bo_eff)
    in_maps = []
    for c in range(NCORES):
        b, g = divmod(c, 2)
        sl = slice(g * OD, (g + 1) * OD)
        bo_c = bo_eff if g == 0 else zeros_bo
        in_maps.append({
            "x_t": np.ascontiguousarray(x[b].T, dtype=f),
            "mem_t": np.ascontiguousarray(memory[b].T, dtype=f),
            "wq_t": np.ascontiguousarray(wqT[:, sl]),
            "wk_t": np.ascontiguousarray(wkT[:, sl]),
            "wv_t": np.ascontiguousarray(wvT[:, sl]),
            "wo_t": np.ascontiguousarray(woT[sl, :]),
            "bq_s": np.ascontiguousarray(bq[sl].astype(f).reshape(OD // P, P).T),
            "bk_s": np.ascontiguousarray(bk[sl].astype(f).reshape(OD // P, P).T),
            "bo_s": np.ascontiguousarray(bo_c.reshape(D // P, P).T),
            "maskb": np.ascontiguousarray(
                np.where(mask[b], np.float32(NEG), np.float32(0.0))
                .astype(f).reshape(NKC, P).T),
        })
    return in_maps


def kernel(x, memory, mask, wq, bq, wk, bk, wv, bv, wo, bo, **run_kwargs):
    x = np.asarray(x, dtype=np.float32)
    memory = np.asarray(memory, dtype=np.float32)
    mask = np.asarray(mask)
    if "nc" not in _cache:
        _cache["nc"] = _build()
    nc = _cache["nc"]
    in_maps = _prep_inputs(x, memory, mask, wq, bq, wk, bk, wv, bv, wo, bo)
    res = run_bass_kernel_spmd(nc, in_maps, list(range(NCORES)), **run_kwargs)
    out = np.empty((B, S, D), dtype=np.float32)
    for b in range(B):
        part = res.results[2 * b]["out_t"] + res.results[2 * b + 1]["out_t"]
        out[b] = part.T
    if run_kwargs:
        _cache["last_results"] = res
    return out


# revision 7
# speedup vs baseline: 1.1387x; 1.1387x over previous
"""Multi-head cross-attention (B=4, S=2048, D=1024, H=16) on 8 Trainium2 cores.

Sharding: hybrid data/tensor parallel. Core c handles batch b = c//2 and
head-group g = c%2 (8 of the 16 heads, i.e. 512 of the 1024 q/k/v dims).
Each core computes a partial out-projection over its 512 attention dims;
the host sums the two partials per batch (the "all-reduce after out_lin"
of the tensor-parallel split, done on host since pairs share a batch).

Per-core kernel (all matmuls in float32r = rounded-fp32 PE fast path):
  Q.T = wq_t.T @ x_t   (+bq)      [512, 2048]   (o on partitions)
  K.T = wk_t.T @ mem_t (+bk)      [512, 2048]
  V   = mem_t.T @ wv_t            [2048, 512] stored as v_aug [*, 8, 65]
                                  with a ones column per head (denominator)
  per head h, query-half qh:
    S.T[k,q] = K_h @ Q_h.T                (K=64 contraction)
    P.T      = exp(0.125*S.T + mask_bias) (ACT, bias is per-key partition)
    AV.T     = [V_h|1].T @ P.T  -> [65, 1024] PSUM accum over k-chunks
    attn.T   = AV.T[0:64] * recip(AV.T[64])  (Newton-refined reciprocal)
  out.T = wo_t.T @ attn.T (+bo_eff on core g=0)  [1024, 2048] partial

bv is folded into bo on the host: out = attn@wo.T + (bo + wo@bv) because
softmax rows sum to 1. The key-padding mask enters as an additive
per-partition bias in the exp activation (exact, and free).
"""

import numpy as np

import concourse.bacc as bacc
import concourse.mybir as mybir
from concourse import tile
from concourse.bass_utils import run_bass_kernel_spmd

F32 = mybir.dt.float32
F32R = mybir.dt.float32r
F16 = mybir.dt.float16
AF = mybir.ActivationFunctionType

B, S, D = 4, 2048, 1024
H, HD = 16, 64
NCORES = 8
NH = 8          # heads per core
OD = NH * HD    # 512 attention dims per core
P = 128
NDC = D // P    # 8 d-chunks
NKC = S // P    # 16 key chunks
NEG = -1.0e30

_cache = {}


def _build():
    nc = bacc.Bacc(None, target_bir_lowering=False, debug=False)

    x_t = nc.dram_tensor("x_t", [D, S], F16, kind="ExternalInput").ap()
    mem_t = nc.dram_tensor("mem_t", [D, S], F16, kind="ExternalInput").ap()
    wq_t = nc.dram_tensor("wq_t", [D, OD], F16, kind="ExternalInput").ap()
    wk_t = nc.dram_tensor("wk_t", [D, OD], F16, kind="ExternalInput").ap()
    wv_t = nc.dram_tensor("wv_t", [D, OD], F16, kind="ExternalInput").ap()
    wo_t = nc.dram_tensor("wo_t", [OD, D], F16, kind="ExternalInput").ap()
    bq_s = nc.dram_tensor("bq_s", [P, OD // P], F32, kind="ExternalInput").ap()
    bk_s = nc.dram_tensor("bk_s", [P, OD // P], F32, kind="ExternalInput").ap()
    bo_s = nc.dram_tensor("bo_s", [P, D // P], F32, kind="ExternalInput").ap()
    maskb = nc.dram_tensor("maskb", [P, NKC], F32, kind="ExternalInput").ap()
    out_t = nc.dram_tensor("out_t", [D, S], F32, kind="ExternalOutput").ap()

    x_c = x_t.rearrange("(c p) s -> c p s", p=P)
    m_c = mem_t.rearrange("(c p) s -> c p s", p=P)
    wq_c = wq_t.rearrange("(c p) o -> c p o", p=P)
    wk_c = wk_t.rearrange("(c p) o -> c p o", p=P)
    wv_c = wv_t.rearrange("(c p) o -> c p o", p=P)
    wo_c = wo_t.rearrange("(c p) o -> c p o", p=P)

    from contextlib import ExitStack
    with tile.TileContext(nc) as tc, ExitStack() as ctx:
        q_pool = ctx.enter_context(tc.tile_pool(name="qt", bufs=1))
        k_pool = ctx.enter_context(tc.tile_pool(name="kt", bufs=1))
        v_pool = ctx.enter_context(tc.tile_pool(name="va", bufs=1))
        a_pool = ctx.enter_context(tc.tile_pool(name="at", bufs=1))
        c_pool = ctx.enter_context(tc.tile_pool(name="cst", bufs=1))
        proj_ctx = ExitStack()
        xm_pool = proj_ctx.enter_context(tc.tile_pool(name="xm", bufs=8))
        w_pool = proj_ctx.enter_context(tc.tile_pool(name="wt", bufs=3))
        if True:
            # ---- constants ----
            bq_sb = c_pool.tile([P, OD // P], F32, tag="bq")
            bk_sb = c_pool.tile([P, OD // P], F32, tag="bk")
            bo_sb = c_pool.tile([P, D // P], F32, tag="bo")
            mk_sb = c_pool.tile([P, NKC], F32, tag="mk")
            nc.sync.dma_start(out=bq_sb[:], in_=bq_s[:])
            nc.sync.dma_start(out=bk_sb[:], in_=bk_s[:])
            nc.sync.dma_start(out=bo_sb[:], in_=bo_s[:])
            nc.sync.dma_start(out=mk_sb[:], in_=maskb[:])
            ones_f = c_pool.tile([P, NH], F32, tag="onef")
            nc.vector.memset(ones_f[:], 1.0)
            ones_r = c_pool.tile([P, NH], F16, tag="oner")
            nc.vector.tensor_copy(ones_r[:], ones_f[:])

            qT = [q_pool.tile([P, S], F16, tag=f"q{m}", name=f"q{m}")
                  for m in range(OD // P)]
            kT = [k_pool.tile([P, S], F16, tag=f"k{m}", name=f"k{m}")
                  for m in range(OD // P)]
            v_aug = [v_pool.tile([P, NH, 65], F16, tag=f"v{st}", name=f"v{st}")
                     for st in range(NKC)]

            HS = S // 2  # column-block of 1024 tokens
            with tc.tile_pool(name="pj", bufs=4, space="PSUM") as pj_pool:
                # ---- phases 1+2+3 blocked over token halves ----
                for nb in range(2):
                    c_sl = slice(nb * HS, (nb + 1) * HS)
                    # phase 1: Q.T over this query half
                    x_tiles = []
                    for i in range(NDC):
                        t = xm_pool.tile([P, HS], F16, tag="xm", name="xt")
                        nc.sync.dma_start(out=t[:], in_=x_c[i, :, c_sl])
                        x_tiles.append(t)
                    for m in range(OD // P):
                        wq_tiles = []
                        for i in range(NDC):
                            wt = w_pool.tile([P, P], F16, tag="w", name="wqt", bufs=10)
                            nc.sync.dma_start(
                                out=wt[:], in_=wq_c[i, :, m * P:(m + 1) * P])
                            wq_tiles.append(wt)
                        for n in range(HS // 512):
                            ps = pj_pool.tile([P, 512], F32, tag="pj", name="psq")
                            for i in range(NDC):
                                nc.tensor.matmul(
                                    ps[:], wq_tiles[i][:],
                                    x_tiles[i][:, n * 512:(n + 1) * 512],
                                    start=(i == 0), stop=(i == NDC - 1),
                                )
                            nc.vector.tensor_scalar_add(
                                qT[m][:, nb * HS + n * 512:nb * HS + (n + 1) * 512],
                                ps[:], bq_sb[:, m:m + 1],
                            )
                    # phase 2: K.T over this key half
                    m_tiles = []
                    for i in range(NDC):
                        t = xm_pool.tile([P, HS], F16, tag="xm", name="mt")
                        nc.sync.dma_start(out=t[:], in_=m_c[i, :, c_sl])
                        m_tiles.append(t)
                    for m in range(OD // P):
                        wk_tiles = []
                        for i in range(NDC):
                            wt = w_pool.tile([P, P], F16, tag="w", name="wkt", bufs=10)
                            nc.sync.dma_start(
                                out=wt[:], in_=wk_c[i, :, m * P:(m + 1) * P])
                            wk_tiles.append(wt)
                        for n in range(HS // 512):
                            ps = pj_pool.tile([P, 512], F32, tag="pj", name="psk")
                            for i in range(NDC):
                                nc.tensor.matmul(
                                    ps[:], wk_tiles[i][:],
                                    m_tiles[i][:, n * 512:(n + 1) * 512],
                                    start=(i == 0), stop=(i == NDC - 1),
                                )
                            nc.vector.tensor_scalar_add(
                                kT[m][:, nb * HS + n * 512:nb * HS + (n + 1) * 512],
                                ps[:], bk_sb[:, m:m + 1],
                            )
                    # phase 3: V for this key half
                    wv_tiles = []
                    for i in range(NDC):
                        wt = w_pool.tile([P, OD], F16, tag="wv", name="wvt", bufs=8)
                        nc.sync.dma_start(out=wt[:], in_=wv_c[i])
                        wv_tiles.append(wt)
                    for sti in range(HS // P):
                        st = nb * (HS // P) + sti
                        ps = pj_pool.tile([P, 512], F32, tag="pj", name="psv")
                        for i in range(NDC):
                            nc.tensor.matmul(
                                ps[:], m_tiles[i][:, sti * P:(sti + 1) * P],
                                wv_tiles[i][:],
                                start=(i == 0), stop=(i == NDC - 1),
                            )
                        nc.vector.tensor_copy(
                            v_aug[st][:, :, 0:64],
                            ps[:].rearrange("p (h d) -> p h d", h=NH),
                        )
                        nc.vector.tensor_copy(
                            v_aug[st][:, :, 64:65], ones_r[:].unsqueeze(2))

            proj_ctx.close()

            # ---- phase 4: attention, head-by-head ----
            attn = [a_pool.tile([P, S], F16, tag=f"a{m}", name=f"a{m}") for m in range(OD // P)]
            with (
                tc.tile_pool(name="es", bufs=3) as e_pool,
                tc.tile_pool(name="nrm", bufs=1) as n_pool,
                tc.tile_pool(name="lg", bufs=2, space="PSUM") as lg_pool,
                tc.tile_pool(name="av", bufs=2, space="PSUM") as av_pool,
            ):
                for qh in range(2):
                    q_sl = slice(qh * 1024, (qh + 1) * 1024)
                    for h in range(NH):
                        mt, ro = h // 2, 64 * (h % 2)
                        av = av_pool.tile([65, 1024], F32, tag="av")
                        for kc in range(NKC):
                            lg = lg_pool.tile([P, 1024], F32, tag="lg")
                            for n in range(2):
                                nc.tensor.matmul(
                                    lg[:, n * 512:(n + 1) * 512],
                                    kT[mt][ro:ro + 64, kc * P:(kc + 1) * P],
                                    qT[mt][ro:ro + 64,
                                           qh * 1024 + n * 512:
                                           qh * 1024 + (n + 1) * 512],
                                    start=True, stop=True,
                                )
                            es = e_pool.tile([P, 1024], F16, tag="es")
                            nc.scalar.activation(
                                es[:], lg[:], AF.Exp,
                                bias=mk_sb[:, kc:kc + 1], scale=0.125,
                            )
                            for n in range(2):
                                nc.tensor.matmul(
                                    av[:, n * 512:(n + 1) * 512],
                                    v_aug[kc][:, h, :],
                                    es[:, n * 512:(n + 1) * 512],
                                    start=(kc == 0), stop=(kc == NKC - 1),
                                )
                        # normalize: attn = av[0:64] * recip(av[64])
                        r0 = n_pool.tile([1, 1024], F32, tag="r0")
                        bc = n_pool.tile([64, 1024], F32, tag="bc")
                        nc.vector.reciprocal(r0[:], av[64:65, :])
                        nc.gpsimd.partition_broadcast(bc[:], r0[:])
                        nc.vector.tensor_mul(
                            attn[mt][ro:ro + 64, q_sl], av[0:64, :], bc[:],
                        )

            # ---- phase 5: out.T = wo_t.T @ attn.T (+bo_eff) ----
            with (
                tc.tile_pool(name="wt2", bufs=3) as w_pool,
                tc.tile_pool(name="ev", bufs=3) as o_pool,
                tc.tile_pool(name="po", bufs=4, space="PSUM") as po_pool,
            ):
                for m in range(D // P):
                    wo_tiles = []
                    for i in range(OD // P):
                        wt = w_pool.tile([P, P], F16, tag="w", bufs=6)
                        nc.sync.dma_start(out=wt[:], in_=wo_c[i, :, m * P:(m + 1) * P])
                        wo_tiles.append(wt)
                    for n in range(S // 512):
                        ps = po_pool.tile([P, 512], F32, tag="po")
                        for i in range(OD // P):
                            nc.tensor.matmul(
                                ps[:], wo_tiles[i][:],
                                attn[i][:, n * 512:(n + 1) * 512],
                                start=(i == 0), stop=(i == OD // P - 1),
                            )
                        ev = o_pool.tile([P, 512], F32, tag="ev")
                        nc.vector.tensor_scalar_add(ev[:], ps[:], bo_sb[:, m:m + 1])
                        nc.sync.dma_start(
                            out=out_t[m * P:(m + 1) * P, n * 512:(n + 1) * 512],
                            in_=ev[:],
                        )

    nc.compile()
    return nc


def _prep_inputs(x, memory, mask, wq, bq, wk, bk, wv, bv, wo, bo):
    f = np.float32
    h = np.float16
    wqT = np.ascontiguousarray(wq.T, dtype=f)
    wkT = np.ascontiguousarray(wk.T, dtype=f)
    wvT = np.ascontiguousarray(wv.T, dtype=f)
    woT = np.ascontiguousarray(wo.T, dtype=f)
    bo_eff = (bo.astype(f) + wo.astype(f) @ bv.astype(f))
    zeros_bo = np.zeros_like(bo_eff)
    in_maps = []
    for c in range(NCORES):
        b, g = divmod(c, 2)
        sl = slice(g * OD, (g + 1) * OD)
        bo_c = bo_eff if g == 0 else zeros_bo
        in_maps.append({
            "x_t": np.ascontiguousarray(x[b].T, dtype=h),
            "mem_t": np.ascontiguousarray(memory[b].T, dtype=h),
            "wq_t": np.ascontiguousarray(wqT[:, sl]).astype(h),
            "wk_t": np.ascontiguousarray(wkT[:, sl]).astype(h),
            "wv_t": np.ascontiguousarray(wvT[:, sl]).astype(h),
            "wo_t": np.ascontiguousarray(woT[sl, :]).astype(h),
            "bq_s": np.ascontiguousarray(bq[sl].astype(f).reshape(OD // P, P).T),
            "bk_s": np.ascontiguousarray(bk[sl].astype(f).reshape(OD // P, P).T),
            "bo_s": np.ascontiguousarray(bo_c.reshape(D // P, P).T),
            "maskb": np.ascontiguousarray(
                np.where(mask[b], np.float32(NEG), np.float32(0.0))
                .astype(f).reshape(NKC, P).T),
        })
    return in_maps


def kernel(x, memory, mask, wq, bq, wk, bk, wv, bv, wo, bo, **run_kwargs):
    x = np.asarray(x, dtype=np.float32)
    memory = np.asarray(memory, dtype=np.float32)
    mask = np.asarray(mask)
    if "nc" not in _cache:
        _cache["nc"] = _build()
    nc = _cache["nc"]
    in_maps = _prep_inputs(x, memory, mask, wq, bq, wk, bk, wv, bv, wo, bo)
    res = run_bass_kernel_spmd(nc, in_maps, list(range(NCORES)), **run_kwargs)
    out = np.empty((B, S, D), dtype=np.float32)
    for b in range(B):
        part = res.results[2 * b]["out_t"] + res.results[2 * b + 1]["out_t"]
        out[b] = part.T
    if run_kwargs:
        _cache["last_results"] = res
    return out


# revision 8
# speedup vs baseline: 1.5953x; 1.4011x over previous
"""Multi-head cross-attention (B=4, S=2048, D=1024, H=16) on 8 Trainium2 cores.

Sharding: hybrid data/tensor parallel. Core c handles batch b = c//2 and
head-group g = c%2 (8 of the 16 heads, i.e. 512 of the 1024 q/k/v dims).
Each core computes a partial out-projection over its 512 attention dims;
the host sums the two partials per batch (the "all-reduce after out_lin"
of the tensor-parallel split, done on host since pairs share a batch).

Per-core kernel (all matmuls in float32r = rounded-fp32 PE fast path):
  Q.T = wq_t.T @ x_t   (+bq)      [512, 2048]   (o on partitions)
  K.T = wk_t.T @ mem_t (+bk)      [512, 2048]
  V   = mem_t.T @ wv_t            [2048, 512] stored as v_aug [*, 8, 65]
                                  with a ones column per head (denominator)
  per head h, query-half qh:
    S.T[k,q] = K_h @ Q_h.T                (K=64 contraction)
    P.T      = exp(0.125*S.T + mask_bias) (ACT, bias is per-key partition)
    AV.T     = [V_h|1].T @ P.T  -> [65, 1024] PSUM accum over k-chunks
    attn.T   = AV.T[0:64] * recip(AV.T[64])  (Newton-refined reciprocal)
  out.T = wo_t.T @ attn.T (+bo_eff on core g=0)  [1024, 2048] partial

bv is folded into bo on the host: out = attn@wo.T + (bo + wo@bv) because
softmax rows sum to 1. The key-padding mask enters as an additive
per-partition bias in the exp activation (exact, and free).
"""

import numpy as np

import concourse.bacc as bacc
import concourse.mybir as mybir
from concourse import tile
from concourse.bass_utils import run_bass_kernel_spmd

F32 = mybir.dt.float32
F32R = mybir.dt.float32r
F16 = mybir.dt.float16
AF = mybir.ActivationFunctionType

B, S, D = 4, 2048, 1024
H, HD = 16, 64
NCORES = 8
NH = 8          # heads per core
OD = NH * HD    # 512 attention dims per core
P = 128
NDC = D // P    # 8 d-chunks
NKC = S // P    # 16 key chunks
NEG = -1.0e30

_cache = {}


def _build():
    nc = bacc.Bacc(None, target_bir_lowering=False, debug=False)

    x_t = nc.dram_tensor("x_t", [D, S], F16, kind="ExternalInput").ap()
    mem_t = nc.dram_tensor("mem_t", [D, S], F16, kind="ExternalInput").ap()
    wq_t = nc.dram_tensor("wq_t", [D, OD], F16, kind="ExternalInput").ap()
    wk_t = nc.dram_tensor("wk_t", [D, OD], F16, kind="ExternalInput").ap()
    wv_t = nc.dram_tensor("wv_t", [D, OD], F16, kind="ExternalInput").ap()
    wo_t = nc.dram_tensor("wo_t", [OD, D], F16, kind="ExternalInput").ap()
    bq_s = nc.dram_tensor("bq_s", [P, OD // P], F32, kind="ExternalInput").ap()
    bk_s = nc.dram_tensor("bk_s", [P, OD // P], F32, kind="ExternalInput").ap()
    bo_s = nc.dram_tensor("bo_s", [P, D // P], F32, kind="ExternalInput").ap()
    maskb = nc.dram_tensor("maskb", [P, NKC], F32, kind="ExternalInput").ap()
    out_t = nc.dram_tensor("out_t", [D, S], F32, kind="ExternalOutput").ap()

    x_c = x_t.rearrange("(c p) s -> c p s", p=P)
    m_c = mem_t.rearrange("(c p) s -> c p s", p=P)
    wq_c = wq_t.rearrange("(c p) o -> c p o", p=P)
    wk_c = wk_t.rearrange("(c p) o -> c p o", p=P)
    wv_c = wv_t.rearrange("(c p) o -> c p o", p=P)
    wo_c = wo_t.rearrange("(c p) o -> c p o", p=P)

    from contextlib import ExitStack
    with tile.TileContext(nc) as tc, ExitStack() as ctx:
        q_pool = ctx.enter_context(tc.tile_pool(name="qt", bufs=1))
        k_pool = ctx.enter_context(tc.tile_pool(name="kt", bufs=1))
        v_pool = ctx.enter_context(tc.tile_pool(name="va", bufs=1))
        a_pool = ctx.enter_context(tc.tile_pool(name="at", bufs=1))
        c_pool = ctx.enter_context(tc.tile_pool(name="cst", bufs=1))
        proj_ctx = ExitStack()
        xm_pool = proj_ctx.enter_context(tc.tile_pool(name="xm", bufs=8))
        w_pool = proj_ctx.enter_context(tc.tile_pool(name="wt", bufs=3))
        if True:
            # ---- constants ----
            bq_sb = c_pool.tile([P, OD // P], F32, tag="bq")
            bk_sb = c_pool.tile([P, OD // P], F32, tag="bk")
            bo_sb = c_pool.tile([P, D // P], F32, tag="bo")
            mk_sb = c_pool.tile([P, NKC], F32, tag="mk")
            nc.sync.dma_start(out=bq_sb[:], in_=bq_s[:])
            nc.sync.dma_start(out=bk_sb[:], in_=bk_s[:])
            nc.sync.dma_start(out=bo_sb[:], in_=bo_s[:])
            nc.sync.dma_start(out=mk_sb[:], in_=maskb[:])
            ones_f = c_pool.tile([P, NH], F32, tag="onef")
            nc.vector.memset(ones_f[:], 1.0)
            ones_r = c_pool.tile([P, NH], F16, tag="oner")
            nc.vector.tensor_copy(ones_r[:], ones_f[:])

            qT = [q_pool.tile([P, S], F16, tag=f"q{m}", name=f"q{m}")
                  for m in range(OD // P)]
            kT = [k_pool.tile([P, S], F16, tag=f"k{h}", name=f"k{h}")
                  for h in range(NH)]
            for h in range(NH):
                ro = 64 * (h % 2)
                nc.vector.memset(kT[h][64 - ro:128 - ro, :], 0.0)
            v_aug = [v_pool.tile([P, 9, 65], F16, tag=f"v{st}", name=f"v{st}")
                     for st in range(NKC)]
            for st in range(NKC):
                nc.vector.memset(v_aug[st][:, 8, :], 0.0)

            HS = S // 2  # column-block of 1024 tokens
            with tc.tile_pool(name="pj", bufs=4, space="PSUM") as pj_pool:
                # ---- phases 1+2+3 blocked over token halves ----
                for nb in range(2):
                    c_sl = slice(nb * HS, (nb + 1) * HS)
                    # phase 1: Q.T over this query half
                    x_tiles = []
                    for i in range(NDC):
                        t = xm_pool.tile([P, HS], F16, tag="xm", name="xt")
                        nc.sync.dma_start(out=t[:], in_=x_c[i, :, c_sl])
                        x_tiles.append(t)
                    for m in range(OD // P):
                        wq_tiles = []
                        for i in range(NDC):
                            wt = w_pool.tile([P, P], F16, tag="w", name="wqt", bufs=10)
                            nc.sync.dma_start(
                                out=wt[:], in_=wq_c[i, :, m * P:(m + 1) * P])
                            wq_tiles.append(wt)
                        for n in range(HS // 512):
                            ps = pj_pool.tile([P, 512], F32, tag="pj", name="psq")
                            for i in range(NDC):
                                nc.tensor.matmul(
                                    ps[:], wq_tiles[i][:],
                                    x_tiles[i][:, n * 512:(n + 1) * 512],
                                    start=(i == 0), stop=(i == NDC - 1),
                                )
                            nc.vector.tensor_scalar_add(
                                qT[m][:, nb * HS + n * 512:nb * HS + (n + 1) * 512],
                                ps[:], bq_sb[:, m:m + 1],
                            )
                    # phase 2: K.T over this key half
                    m_tiles = []
                    for i in range(NDC):
                        t = xm_pool.tile([P, HS], F16, tag="xm", name="mt")
                        nc.sync.dma_start(out=t[:], in_=m_c[i, :, c_sl])
                        m_tiles.append(t)
                    for m in range(OD // P):
                        wk_tiles = []
                        for i in range(NDC):
                            wt = w_pool.tile([P, P], F16, tag="w", name="wkt", bufs=10)
                            nc.sync.dma_start(
                                out=wt[:], in_=wk_c[i, :, m * P:(m + 1) * P])
                            wk_tiles.append(wt)
                        for n in range(HS // 512):
                            ps = pj_pool.tile([P, 512], F32, tag="pj", name="psk")
                            for i in range(NDC):
                                nc.tensor.matmul(
                                    ps[:], wk_tiles[i][:],
                                    m_tiles[i][:, n * 512:(n + 1) * 512],
                                    start=(i == 0), stop=(i == NDC - 1),
                                )
                            csl = slice(nb * HS + n * 512, nb * HS + (n + 1) * 512)
                            nc.vector.tensor_scalar_add(
                                kT[2 * m][0:64, csl], ps[0:64, :],
                                bk_sb[0:64, m:m + 1],
                            )
                            nc.vector.tensor_scalar_add(
                                kT[2 * m + 1][64:128, csl], ps[64:128, :],
                                bk_sb[64:128, m:m + 1],
                            )
                    # phase 3: V for this key half
                    wv_tiles = []
                    for i in range(NDC):
                        wt = w_pool.tile([P, OD], F16, tag="wv", name="wvt", bufs=8)
                        nc.sync.dma_start(out=wt[:], in_=wv_c[i])
                        wv_tiles.append(wt)
                    for sti in range(HS // P):
                        st = nb * (HS // P) + sti
                        ps = pj_pool.tile([P, 512], F32, tag="pj", name="psv")
                        for i in range(NDC):
                            nc.tensor.matmul(
                                ps[:], m_tiles[i][:, sti * P:(sti + 1) * P],
                                wv_tiles[i][:],
                                start=(i == 0), stop=(i == NDC - 1),
                            )
                        nc.vector.tensor_copy(
                            v_aug[st][:, 0:NH, 0:64],
                            ps[:].rearrange("p (h d) -> p h d", h=NH),
                        )
                        nc.vector.tensor_copy(
                            v_aug[st][:, 0:NH, 64:65], ones_r[:].unsqueeze(2))

            proj_ctx.close()

            # ---- phase 4: attention, head-by-head ----
            attn = [a_pool.tile([P, S], F16, tag=f"a{m}", name=f"a{m}") for m in range(OD // P)]
            with (
                tc.tile_pool(name="es", bufs=3) as e_pool,
                tc.tile_pool(name="nrm", bufs=1) as n_pool,
                tc.tile_pool(name="lg", bufs=2, space="PSUM") as lg_pool,
                tc.tile_pool(name="av", bufs=2, space="PSUM") as av_pool,
            ):
                for qh in range(2):
                    q_sl = slice(qh * 1024, (qh + 1) * 1024)
                    for h in range(NH):
                        mt = h // 2
                        av = av_pool.tile([P, 1024], F32, tag="av")
                        for kc in range(NKC):
                            lg = lg_pool.tile([P, 1024], F32, tag="lg")
                            for n in range(2):
                                nc.tensor.matmul(
                                    lg[:, n * 512:(n + 1) * 512],
                                    kT[h][:, kc * P:(kc + 1) * P],
                                    qT[mt][:,
                                           qh * 1024 + n * 512:
                                           qh * 1024 + (n + 1) * 512],
                                    start=True, stop=True,
                                )
                            es = e_pool.tile([P, 1024], F16, tag="es")
                            nc.scalar.activation(
                                es[:], lg[:], AF.Exp,
                                bias=mk_sb[:, kc:kc + 1], scale=0.125,
                            )
                            va_flat = v_aug[kc][:].rearrange("p h d -> p (h d)")
                            for n in range(2):
                                nc.tensor.matmul(
                                    av[:, n * 512:(n + 1) * 512],
                                    va_flat[:, 65 * h:65 * h + 128],
                                    es[:, n * 512:(n + 1) * 512],
                                    start=(kc == 0), stop=(kc == NKC - 1),
                                )
                        # normalize: attn = av[0:64] * recip(av[64])
                        r0 = n_pool.tile([1, 1024], F32, tag="r0")
                        bc = n_pool.tile([64, 1024], F32, tag="bc")
                        ro = 64 * (h % 2)
                        nc.vector.reciprocal(r0[:], av[64:65, :])
                        nc.gpsimd.partition_broadcast(bc[:], r0[:])
                        nc.vector.tensor_mul(
                            attn[mt][ro:ro + 64, q_sl], av[0:64, :], bc[:],
                        )

            # ---- phase 5: out.T = wo_t.T @ attn.T (+bo_eff) ----
            with (
                tc.tile_pool(name="wt2", bufs=3) as w_pool,
                tc.tile_pool(name="ev", bufs=3) as o_pool,
                tc.tile_pool(name="po", bufs=4, space="PSUM") as po_pool,
            ):
                for m in range(D // P):
                    wo_tiles = []
                    for i in range(OD // P):
                        wt = w_pool.tile([P, P], F16, tag="w", bufs=6)
                        nc.sync.dma_start(out=wt[:], in_=wo_c[i, :, m * P:(m + 1) * P])
                        wo_tiles.append(wt)
                    for n in range(S // 512):
                        ps = po_pool.tile([P, 512], F32, tag="po")
                        for i in range(OD // P):
                            nc.tensor.matmul(
                                ps[:], wo_tiles[i][:],
                                attn[i][:, n * 512:(n + 1) * 512],
                                start=(i == 0), stop=(i == OD // P - 1),
                            )
                        ev = o_pool.tile([P, 512], F32, tag="ev")
                        nc.vector.tensor_scalar_add(ev[:], ps[:], bo_sb[:, m:m + 1])
                        nc.sync.dma_start(
                            out=out_t[m * P:(m + 1) * P, n * 512:(n + 1) * 512],
                            in_=ev[:],
                        )

    nc.compile()
    return nc


def _prep_inputs(x, memory, mask, wq, bq, wk, bk, wv, bv, wo, bo):
    f = np.float32
    h = np.float16
    wqT = np.ascontiguousarray(wq.T, dtype=f)
    wkT = np.ascontiguousarray(wk.T, dtype=f)
    wvT = np.ascontiguousarray(wv.T, dtype=f)
    woT = np.ascontiguousarray(wo.T, dtype=f)
    bo_eff = (bo.astype(f) + wo.astype(f) @ bv.astype(f))
    zeros_bo = np.zeros_like(bo_eff)
    in_maps = []
    for c in range(NCORES):
        b, g = divmod(c, 2)
        sl = slice(g * OD, (g + 1) * OD)
        bo_c = bo_eff if g == 0 else zeros_bo
        in_maps.append({
            "x_t": np.ascontiguousarray(x[b].T, dtype=h),
            "mem_t": np.ascontiguousarray(memory[b].T, dtype=h),
            "wq_t": np.ascontiguousarray(wqT[:, sl]).astype(h),
            "wk_t": np.ascontiguousarray(wkT[:, sl]).astype(h),
            "wv_t": np.ascontiguousarray(wvT[:, sl]).astype(h),
            "wo_t": np.ascontiguousarray(woT[sl, :]).astype(h),
            "bq_s": np.ascontiguousarray(bq[sl].astype(f).reshape(OD // P, P).T),
            "bk_s": np.ascontiguousarray(bk[sl].astype(f).reshape(OD // P, P).T),
            "bo_s": np.ascontiguousarray(bo_c.reshape(D // P, P).T),
            "maskb": np.ascontiguousarray(
                np.where(mask[b], np.float32(NEG), np.float32(0.0))
                .astype(f).reshape(NKC, P).T),
        })
    return in_maps


def kernel(x, memory, mask, wq, bq, wk, bk, wv, bv, wo, bo, **run_kwargs):
    x = np.asarray(x, dtype=np.float32)
    memory = np.asarray(memory, dtype=np.float32)
    mask = np.asarray(mask)
    if "nc" not in _cache:
        _cache["nc"] = _build()
    nc = _cache["nc"]
    in_maps = _prep_inputs(x, memory, mask, wq, bq, wk, bk, wv, bv, wo, bo)
    res = run_bass_kernel_spmd(nc, in_maps, list(range(NCORES)), **run_kwargs)
    out = np.empty((B, S, D), dtype=np.float32)
    for b in range(B):
        part = res.results[2 * b]["out_t"] + res.results[2 * b + 1]["out_t"]
        out[b] = part.T
    if run_kwargs:
        _cache["last_results"] = res
    return out


# revision 10
# speedup vs baseline: 1.6220x; 1.0167x over previous
"""Multi-head cross-attention (B=4, S=2048, D=1024, H=16) on 8 Trainium2 cores.

Sharding: hybrid data/tensor parallel. Core c handles batch b = c//2 and
head-group g = c%2 (8 of the 16 heads, i.e. 512 of the 1024 q/k/v dims).
Each core computes a partial out-projection over its 512 attention dims;
the host sums the two partials per batch (the "all-reduce after out_lin"
of the tensor-parallel split, done on host since pairs share a batch).

Per-core kernel (all matmuls in float32r = rounded-fp32 PE fast path):
  Q.T = wq_t.T @ x_t   (+bq)      [512, 2048]   (o on partitions)
  K.T = wk_t.T @ mem_t (+bk)      [512, 2048]
  V   = mem_t.T @ wv_t            [2048, 512] stored as v_aug [*, 8, 65]
                                  with a ones column per head (denominator)
  per head h, query-half qh:
    S.T[k,q] = K_h @ Q_h.T                (K=64 contraction)
    P.T      = exp(0.125*S.T + mask_bias) (ACT, bias is per-key partition)
    AV.T     = [V_h|1].T @ P.T  -> [65, 1024] PSUM accum over k-chunks
    attn.T   = AV.T[0:64] * recip(AV.T[64])  (Newton-refined reciprocal)
  out.T = wo_t.T @ attn.T (+bo_eff on core g=0)  [1024, 2048] partial

bv is folded into bo on the host: out = attn@wo.T + (bo + wo@bv) because
softmax rows sum to 1. The key-padding mask enters as an additive
per-partition bias in the exp activation (exact, and free).
"""

import numpy as np

import concourse.bacc as bacc
import concourse.mybir as mybir
from concourse import tile
from concourse.bass_utils import run_bass_kernel_spmd

F32 = mybir.dt.float32
F32R = mybir.dt.float32r
F16 = mybir.dt.float16
AF = mybir.ActivationFunctionType

B, S, D = 4, 2048, 1024
H, HD = 16, 64
NCORES = 8
NH = 8          # heads per core
OD = NH * HD    # 512 attention dims per core
P = 128
NDC = D // P    # 8 d-chunks
NKC = S // P    # 16 key chunks
NEG = -1.0e30

_cache = {}


def _build():
    nc = bacc.Bacc(None, target_bir_lowering=False, debug=False)

    x_t = nc.dram_tensor("x_t", [D, S], F16, kind="ExternalInput").ap()
    mem_t = nc.dram_tensor("mem_t", [D, S], F16, kind="ExternalInput").ap()
    wq_t = nc.dram_tensor("wq_t", [D, OD], F16, kind="ExternalInput").ap()
    wk_t = nc.dram_tensor("wk_t", [D, OD], F16, kind="ExternalInput").ap()
    wv_t = nc.dram_tensor("wv_t", [D, OD], F16, kind="ExternalInput").ap()
    wo_t = nc.dram_tensor("wo_t", [OD, D], F16, kind="ExternalInput").ap()
    bq_s = nc.dram_tensor("bq_s", [P, OD // P], F32, kind="ExternalInput").ap()
    bk_s = nc.dram_tensor("bk_s", [P, OD // P], F32, kind="ExternalInput").ap()
    bo_s = nc.dram_tensor("bo_s", [P, D // P], F32, kind="ExternalInput").ap()
    maskb = nc.dram_tensor("maskb", [P, NKC], F32, kind="ExternalInput").ap()
    out_t = nc.dram_tensor("out_t", [D, S], F32, kind="ExternalOutput").ap()

    x_c = x_t.rearrange("(c p) s -> c p s", p=P)
    m_c = mem_t.rearrange("(c p) s -> c p s", p=P)
    wq_c = wq_t.rearrange("(c p) o -> c p o", p=P)
    wk_c = wk_t.rearrange("(c p) o -> c p o", p=P)
    wv_c = wv_t.rearrange("(c p) o -> c p o", p=P)
    wo_c = wo_t.rearrange("(c p) o -> c p o", p=P)

    NMM = 1024   # moving-dim per matmul (fp16 allows 1024)
    NS = S // NMM

    with tile.TileContext(nc) as tc:
        with (
            tc.tile_pool(name="xm", bufs=8) as xm_pool,
            tc.tile_pool(name="wt", bufs=10) as w_pool,
            tc.tile_pool(name="qt", bufs=1) as q_pool,
            tc.tile_pool(name="kt", bufs=1) as k_pool,
            tc.tile_pool(name="va", bufs=1) as v_pool,
            tc.tile_pool(name="at", bufs=1) as a_pool,
            tc.tile_pool(name="cst", bufs=1) as c_pool,
            tc.tile_pool(name="es", bufs=3) as e_pool,
            tc.tile_pool(name="nrm", bufs=1) as n_pool,
            tc.tile_pool(name="ev", bufs=3) as o_pool,
            tc.tile_pool(name="ps", bufs=2, space="PSUM") as psum_pool,
        ):
            # ---- constants ----
            bq_sb = c_pool.tile([P, OD // P], F32, tag="bq")
            bk_sb = c_pool.tile([P, OD // P], F32, tag="bk")
            bo_sb = c_pool.tile([P, D // P], F32, tag="bo")
            mk_sb = c_pool.tile([P, NKC], F32, tag="mk")
            nc.sync.dma_start(out=bq_sb[:], in_=bq_s[:])
            nc.sync.dma_start(out=bk_sb[:], in_=bk_s[:])
            nc.sync.dma_start(out=bo_sb[:], in_=bo_s[:])
            nc.sync.dma_start(out=mk_sb[:], in_=maskb[:])
            ones_f = c_pool.tile([P, NH], F32, tag="onef")
            nc.vector.memset(ones_f[:], 1.0)
            ones_r = c_pool.tile([P, NH], F16, tag="oner")
            nc.vector.tensor_copy(ones_r[:], ones_f[:])

            # ---- persistent tiles ----
            qT = [q_pool.tile([P, S], F16, tag=f"q{m}", name=f"q{m}")
                  for m in range(OD // P)]
            kT = [k_pool.tile([P, S], F16, tag=f"k{h}", name=f"k{h}")
                  for h in range(NH)]
            for h in range(NH):
                ro = 64 * (h % 2)
                nc.vector.memset(kT[h][64 - ro:128 - ro, :], 0.0)
            v_aug = [v_pool.tile([P, 9, 65], F16, tag=f"v{st}", name=f"v{st}")
                     for st in range(NKC)]
            for st in range(NKC):
                nc.vector.memset(v_aug[st][:, 8, :], 0.0)
            attn = [a_pool.tile([P, S], F16, tag=f"a{m}", name=f"a{m}")
                    for m in range(OD // P)]

            # ---- K.T = wk_t.T @ mem_t (+bk), into zero-padded per-head tiles ----
            m_tiles = []
            for i in range(NDC):
                t = xm_pool.tile([P, S], F16, tag="xm", name="mt")
                nc.sync.dma_start(out=t[:], in_=m_c[i])
                m_tiles.append(t)
            for m in range(OD // P):
                wk_tiles = []
                for i in range(NDC):
                    wt = w_pool.tile([P, P], F16, tag="w", name="wkt", bufs=10)
                    nc.sync.dma_start(out=wt[:], in_=wk_c[i, :, m * P:(m + 1) * P])
                    wk_tiles.append(wt)
                for n in range(NS):
                    csl = slice(n * NMM, (n + 1) * NMM)
                    ps = psum_pool.tile([P, NMM], F32, tag="lg", name="psk")
                    for i in range(NDC):
                        for j in range(2):
                            nc.tensor.matmul(
                                ps[:, j * 512:(j + 1) * 512], wk_tiles[i][:],
                                m_tiles[i][:, n * NMM + j * 512:
                                           n * NMM + (j + 1) * 512],
                                start=(i == 0), stop=(i == NDC - 1),
                            )
                    nc.vector.tensor_scalar_add(
                        kT[2 * m][0:64, csl], ps[0:64, :], bk_sb[0:64, m:m + 1])
                    nc.vector.tensor_scalar_add(
                        kT[2 * m + 1][64:128, csl], ps[64:128, :],
                        bk_sb[64:128, m:m + 1])

            # ---- V (s-major) into v_aug ----
            wv_tiles = []
            for i in range(NDC):
                wt = w_pool.tile([P, OD], F16, tag="wv", name="wvt", bufs=8)
                nc.sync.dma_start(out=wt[:], in_=wv_c[i])
                wv_tiles.append(wt)
            for st in range(NKC):
                ps = psum_pool.tile([P, NMM], F32, tag="lg", name="psv")
                for i in range(NDC):
                    nc.tensor.matmul(
                        ps[:, 0:OD], m_tiles[i][:, st * P:(st + 1) * P],
                        wv_tiles[i][:],
                        start=(i == 0), stop=(i == NDC - 1),
                    )
                nc.vector.tensor_copy(
                    v_aug[st][:, 0:NH, 0:64],
                    ps[:, 0:OD].rearrange("p (h d) -> p h d", h=NH),
                )
                nc.vector.tensor_copy(
                    v_aug[st][:, 0:NH, 64:65], ones_r[:].unsqueeze(2))

            # ---- x loads (reuse xm slots) ----
            x_tiles = []
            for i in range(NDC):
                t = xm_pool.tile([P, S], F16, tag="xm", name="xt")
                nc.sync.dma_start(out=t[:], in_=x_c[i])
                x_tiles.append(t)

            # ---- per head-pair: Q.T projection, then attention ----
            for mt in range(OD // P):
                wq_tiles = []
                for i in range(NDC):
                    wt = w_pool.tile([P, P], F16, tag="w", name="wqt", bufs=10)
                    nc.sync.dma_start(out=wt[:], in_=wq_c[i, :, mt * P:(mt + 1) * P])
                    wq_tiles.append(wt)
                for n in range(NS):
                    csl = slice(n * NMM, (n + 1) * NMM)
                    ps = psum_pool.tile([P, NMM], F32, tag="lg", name="psq")
                    for i in range(NDC):
                        for j in range(2):
                            nc.tensor.matmul(
                                ps[:, j * 512:(j + 1) * 512], wq_tiles[i][:],
                                x_tiles[i][:, n * NMM + j * 512:
                                           n * NMM + (j + 1) * 512],
                                start=(i == 0), stop=(i == NDC - 1),
                            )
                    nc.vector.tensor_scalar_add(
                        qT[mt][:, csl], ps[:], bq_sb[:, mt:mt + 1])

                for h in (2 * mt, 2 * mt + 1):
                    ro = 64 * (h % 2)
                    for qh in range(2):
                        q_sl = slice(qh * 1024, (qh + 1) * 1024)
                        av = psum_pool.tile([P, 1024], F32, tag="av", name="av")
                        for kc in range(NKC):
                            lg = psum_pool.tile([P, 1024], F32, tag="lg", name="lg")
                            for j in range(2):
                                nc.tensor.matmul(
                                    lg[:, j * 512:(j + 1) * 512],
                                    kT[h][:, kc * P:(kc + 1) * P],
                                    qT[mt][:, qh * 1024 + j * 512:
                                            qh * 1024 + (j + 1) * 512],
                                    start=True, stop=True,
                                )
                            es = e_pool.tile([P, 1024], F16, tag="es")
                            nc.scalar.activation(
                                es[:], lg[:], AF.Exp,
                                bias=mk_sb[:, kc:kc + 1], scale=0.125,
                            )
                            va_flat = v_aug[kc][:].rearrange("p h d -> p (h d)")
                            for j in range(2):
                                nc.tensor.matmul(
                                    av[:, j * 512:(j + 1) * 512],
                                    va_flat[:, 65 * h:65 * h + 128],
                                    es[:, j * 512:(j + 1) * 512],
                                    start=(kc == 0), stop=(kc == NKC - 1),
                                )
                        r0 = n_pool.tile([1, 1024], F32, tag="r0")
                        bc = n_pool.tile([64, 1024], F32, tag="bc")
                        nc.vector.reciprocal(r0[:], av[64:65, :])
                        nc.gpsimd.partition_broadcast(bc[:], r0[:])
                        nc.vector.tensor_mul(
                            attn[mt][ro:ro + 64, q_sl], av[0:64, :], bc[:])

            # ---- out.T = wo_t.T @ attn.T (+bo_eff) ----
            for m in range(D // P):
                wo_tiles = []
                for i in range(OD // P):
                    wt = w_pool.tile([P, P], F16, tag="w", name="wot", bufs=10)
                    nc.sync.dma_start(out=wt[:], in_=wo_c[i, :, m * P:(m + 1) * P])
                    wo_tiles.append(wt)
                for n in range(NS):
                    csl = slice(n * NMM, (n + 1) * NMM)
                    ps = psum_pool.tile([P, NMM], F32, tag="av", name="pso")
                    for i in range(OD // P):
                        for j in range(2):
                            nc.tensor.matmul(
                                ps[:, j * 512:(j + 1) * 512], wo_tiles[i][:],
                                attn[i][:, n * NMM + j * 512:
                                        n * NMM + (j + 1) * 512],
                                start=(i == 0), stop=(i == OD // P - 1),
                            )
                    ev = o_pool.tile([P, NMM], F32, tag="ev")
                    nc.vector.tensor_scalar_add(ev[:], ps[:], bo_sb[:, m:m + 1])
                    nc.sync.dma_start(
                        out=out_t[m * P:(m + 1) * P, csl], in_=ev[:])

    nc.compile()
    return nc


def _prep_inputs(x, memory, mask, wq, bq, wk, bk, wv, bv, wo, bo):
    f = np.float32
    h = np.float16
    wqT = np.ascontiguousarray(wq.T, dtype=f)
    wkT = np.ascontiguousarray(wk.T, dtype=f)
    wvT = np.ascontiguousarray(wv.T, dtype=f)
    woT = np.ascontiguousarray(wo.T, dtype=f)
    bo_eff = (bo.astype(f) + wo.astype(f) @ bv.astype(f))
    zeros_bo = np.zeros_like(bo_eff)
    in_maps = []
    for c in range(NCORES):
        b, g = divmod(c, 2)
        sl = slice(g * OD, (g + 1) * OD)
        bo_c = bo_eff if g == 0 else zeros_bo
        in_maps.append({
            "x_t": np.ascontiguousarray(x[b].T, dtype=h),
            "mem_t": np.ascontiguousarray(memory[b].T, dtype=h),
            "wq_t": np.ascontiguousarray(wqT[:, sl]).astype(h),
            "wk_t": np.ascontiguousarray(wkT[:, sl]).astype(h),
            "wv_t": np.ascontiguousarray(wvT[:, sl]).astype(h),
            "wo_t": np.ascontiguousarray(woT[sl, :]).astype(h),
            "bq_s": np.ascontiguousarray(bq[sl].astype(f).reshape(OD // P, P).T),
            "bk_s": np.ascontiguousarray(bk[sl].astype(f).reshape(OD // P, P).T),
            "bo_s": np.ascontiguousarray(bo_c.reshape(D // P, P).T),
            "maskb": np.ascontiguousarray(
                np.where(mask[b], np.float32(NEG), np.float32(0.0))
                .astype(f).reshape(NKC, P).T),
        })
    return in_maps


def kernel(x, memory, mask, wq, bq, wk, bk, wv, bv, wo, bo, **run_kwargs):
    x = np.asarray(x, dtype=np.float32)
    memory = np.asarray(memory, dtype=np.float32)
    mask = np.asarray(mask)
    if "nc" not in _cache:
        _cache["nc"] = _build()
    nc = _cache["nc"]
    in_maps = _prep_inputs(x, memory, mask, wq, bq, wk, bk, wv, bv, wo, bo)
    res = run_bass_kernel_spmd(nc, in_maps, list(range(NCORES)), **run_kwargs)
    out = np.empty((B, S, D), dtype=np.float32)
    for b in range(B):
        part = res.results[2 * b]["out_t"] + res.results[2 * b + 1]["out_t"]
        out[b] = part.T
    if run_kwargs:
        _cache["last_results"] = res
    return out
